# revision 3
# baseline (speedup 1.0000x reference)
"""Trainium2 Bass kernel for nn_AttentionBlock (8-core SPMD, query-row sharded).

Algorithm (per core, rows q = 2048 of x):
  XQ = x @ Wq                      [q, 256]
  YK = y @ Wk, YV = y @ Wv         [4096, 256]
  S^T = YK @ XQ^T  (layout B: keys on partitions, no max subtraction)
  U = exp(S^T / 16), V = 0.1 * relu(S^T / 16)
  G1 = U^T @ [YV | 1], G2 = V^T @ [YV | 1]   (row sums come from ones column)
  out = (G1[:, :256] / Z + G2[:, :256]) / D, Z = G1[:, 256], D = G2[:, 256] + 1
"""

import numpy as np

import concourse.bass as bass
import concourse.mybir as mybir
import concourse.tile as tile
from concourse import bacc
from concourse.bass_utils import run_bass_kernel_spmd
from concourse.masks import make_identity

P = 128
N_CORES = 8
N_FULL, M_CTX, SIN, YDIM, SPROJ = 16384, 4096, 256, 7, 256
Q = N_FULL // N_CORES          # 2048 query rows per core
QT = Q // P                    # 16 q-tiles
KT = M_CTX // P                # 32 k-tiles
CC = SPROJ // P                # 2 contraction chunks (proj dim)
QB = 256                       # q-block width in the main loop
NQB = Q // QB                  # 8 q-blocks
SCALE = 1.0 / 16.0             # 1/sqrt(SPROJ)

MM_MODE = "f32"                # "f32" | "f32r" | "bf16"

F32 = mybir.dt.float32
MM_SB_DT = mybir.dt.bfloat16 if MM_MODE == "bf16" else F32


def _mm(ap):
    """Cast an SBUF operand AP to the matmul dtype."""
    if MM_MODE == "f32r":
        return ap.bitcast(mybir.dt.float32r)
    return ap


def _build():
    nc = bacc.Bacc(
        "TRN2",
        target_bir_lowering=False,
        debug=False,
        num_devices=N_CORES,
    )
    x_d = nc.dram_tensor("x", [Q, SIN], F32, kind="ExternalInput").ap()
    y_d = nc.dram_tensor("y", [M_CTX, YDIM], F32, kind="ExternalInput").ap()
    wq_d = nc.dram_tensor("Wq", [SIN, SPROJ], F32, kind="ExternalInput").ap()
    wk_d = nc.dram_tensor("Wk", [YDIM, SPROJ], F32, kind="ExternalInput").ap()
    wv_d = nc.dram_tensor("Wv", [YDIM, SPROJ], F32, kind="ExternalInput").ap()
    out_d = nc.dram_tensor("out", [Q, SPROJ], F32, kind="ExternalOutput").ap()

    with tile.TileContext(nc) as tc:
        _body(tc, x_d, y_d, wq_d, wk_d, wv_d, out_d)
    nc.compile()
    return nc


def _body(tc, x_d, y_d, wq_d, wk_d, wv_d, out_d):
    nc = tc.nc
    Exp = mybir.ActivationFunctionType.Exp

    with tc.tile_pool(name="persist", bufs=1) as persist:
        ident = persist.tile([P, P], F32, tag="ident")
        make_identity(nc, ident)

        # persistent operand tensors for the main loop
        ykT = persist.tile([P, CC, M_CTX], MM_SB_DT, tag="ykT")   # (YK^T)[c, k]
        yvo = persist.tile([P, KT, SPROJ + 1], MM_SB_DT, tag="yvo")  # [YV|1] per k-tile
        xqT = persist.tile([P, CC, Q], MM_SB_DT, tag="xqT")       # (XQ^T)[c, q]

        # ---------------- preamble ----------------
        with (
            tc.tile_pool(name="pre", bufs=2) as pre,
            tc.tile_pool(name="pre_ps", bufs=2, space="PSUM") as pre_ps,
        ):
            wq_sb = pre.tile([P, CC, SPROJ], F32, tag="wq")
            nc.sync.dma_start(wq_sb[:], wq_d.rearrange("(o p) f -> p o f", p=P))

            wk_sb = pre.tile([P, SPROJ], F32, tag="wk")
            nc.vector.memset(wk_sb[:], 0.0)
            nc.sync.dma_start(wk_sb[:YDIM, :], wk_d)

            wv_sb = pre.tile([P, SPROJ], F32, tag="wv")
            nc.vector.memset(wv_sb[:], 0.0)
            nc.sync.dma_start(wv_sb[:YDIM, :], wv_d)

            y_sb = pre.tile([P, KT, YDIM], F32, tag="y")
            nc.sync.dma_start(y_sb[:], y_d.rearrange("(o p) f -> p o f", p=P))

            # y^T [7(pad 128), 4096] via PE transposes
            yT = pre.tile([P, M_CTX], F32, tag="yT")
            nc.vector.memset(yT[:], 0.0)
            for o in range(KT):
                ps = pre_ps.tile([P, P], F32, tag="tps")
                nc.tensor.transpose(ps[:YDIM, :], y_sb[:, o, :], ident)
                nc.vector.tensor_copy(yT[:YDIM, o * P:(o + 1) * P], ps[:YDIM, :])

            # YK^T chunks: [c-chunk 128, m] = Wk[:, c-chunk].T-contracted with y^T
            for c in range(CC):
                for mb in range(M_CTX // 512):
                    ps = pre_ps.tile([P, 512], F32, tag="mmps")
                    nc.tensor.matmul(
                        ps,
                        lhsT=_mm(wk_sb[:, c * P:(c + 1) * P]),
                        rhs=_mm(yT[:, mb * 512:(mb + 1) * 512]),
                        start=True, stop=True,
                    )
                    nc.vector.tensor_copy(ykT[:, c, mb * 512:(mb + 1) * 512], ps)

            # YVo tiles: [k-tile 128, 257]
            for o in range(KT):
                ps = pre_ps.tile([P, 512], F32, tag="mmps")
                nc.tensor.matmul(
                    ps[:, :SPROJ],
                    lhsT=_mm(yT[:, o * P:(o + 1) * P]),
                    rhs=_mm(wv_sb[:]),
                    start=True, stop=True,
                )
                nc.vector.tensor_copy(yvo[:, o, :SPROJ], ps[:, :SPROJ])
                nc.vector.memset(yvo[:, o, SPROJ:SPROJ + 1], 1.0)

            # x tiles -> x^T chunks via PE transposes
            x_sb = pre.tile([P, QT, SIN], F32, tag="x")
            nc.sync.dma_start(x_sb[:], x_d.rearrange("(o p) f -> p o f", p=P))
            xT = pre.tile([P, CC, Q], F32, tag="xT")
            for t in range(QT):
                for c in range(CC):
                    ps = pre_ps.tile([P, P], F32, tag="tps")
                    nc.tensor.transpose(ps, x_sb[:, t, c * P:(c + 1) * P], ident)
                    nc.vector.tensor_copy(xT[:, c, t * P:(t + 1) * P], ps)

            # XQ^T chunks: accumulate over SIN chunks
            for pj in range(CC):
                for qb4 in range(Q // 512):
                    ps = pre_ps.tile([P, 512], F32, tag="mmps")
                    for ci in range(CC):
                        nc.tensor.matmul(
                            ps,
                            lhsT=_mm(wq_sb[:, ci, pj * P:(pj + 1) * P]),
                            rhs=_mm(xT[:, ci, qb4 * 512:(qb4 + 1) * 512]),
                            start=(ci == 0), stop=(ci == CC - 1),
                        )
                    nc.vector.tensor_copy(xqT[:, pj, qb4 * 512:(qb4 + 1) * 512], ps)

        # ---------------- main loop ----------------
        with (
            tc.tile_pool(name="spool", bufs=3, space="PSUM") as spool,
            tc.tile_pool(name="gpool", bufs=1, space="PSUM") as gpool,
            tc.tile_pool(name="uv", bufs=3) as uvpool,
            tc.tile_pool(name="epi", bufs=2) as epi,
        ):
            for qb in range(NQB):
                q0 = qb * QB
                g = [
                    gpool.tile([P, SPROJ + 1], F32, tag=f"g{i}", name=f"g{i}_{qb}")
                    for i in range(4)
                ]  # g[0], g[1]: G1 for q-sub 0/1; g[2], g[3]: G2

                prev_uv = None
                for kt in range(KT):
                    ps_s = spool.tile([P, QB], F32, tag="s")
                    for ci in range(CC):
                        nc.tensor.matmul(
                            ps_s,
                            lhsT=_mm(ykT[:, ci, kt * P:(kt + 1) * P]),
                            rhs=_mm(xqT[:, ci, q0:q0 + QB]),
                            start=(ci == 0), stop=(ci == CC - 1),
                        )

                    # software-pipelined: issue previous iteration's G matmuls
                    if prev_uv is not None:
                        _g_matmuls(nc, g, yvo, prev_uv, kt - 1)

                    u = uvpool.tile([P, QB], MM_SB_DT, tag="u")
                    nc.scalar.activation(u[:], ps_s[:], Exp, scale=SCALE)
                    v = uvpool.tile([P, QB], MM_SB_DT, tag="v")
                    nc.vector.tensor_scalar(
                        v[:], ps_s[:], 0.1 * SCALE, 0.0,
                        mybir.AluOpType.mult, mybir.AluOpType.max,
                    )
                    prev_uv = (u, v)

                _g_matmuls(nc, g, yvo, prev_uv, KT - 1)

                # epilogue: combine and store [128, 256] per q-sub-tile
                for qs in range(2):
                    g1, g2 = g[qs], g[2 + qs]
                    zinv = epi.tile([P, 1], F32, tag="zinv")
                    nc.vector.reciprocal(zinv[:], g1[:, SPROJ:SPROJ + 1])
                    dp1 = epi.tile([P, 1], F32, tag="dp1")
                    nc.vector.tensor_scalar_add(dp1[:], g2[:, SPROJ:SPROJ + 1], 1.0)
                    dinv = epi.tile([P, 1], F32, tag="dinv")
                    nc.vector.reciprocal(dinv[:], dp1[:])

                    acc = epi.tile([P, SPROJ], F32, tag="acc")
                    nc.vector.tensor_scalar_mul(acc[:], g1[:, :SPROJ], zinv[:])
                    nc.vector.tensor_tensor(
                        acc[:], acc[:], g2[:, :SPROJ], mybir.AluOpType.add
                    )
                    out_t = epi.tile([P, SPROJ], F32, tag="out")
                    nc.vector.tensor_scalar_mul(out_t[:], acc[:], dinv[:])
                    r0 = q0 + qs * P
                    nc.sync.dma_start(out_d[r0:r0 + P, :], out_t[:])


def _g_matmuls(nc, g, yvo, uv, kt):
    u, v = uv
    for qs in range(2):
        for gi, src in ((qs, u), (2 + qs, v)):
            nc.tensor.matmul(
                g[gi],
                lhsT=_mm(src[:, qs * P:(qs + 1) * P]),
                rhs=_mm(yvo[:, kt, :]),
                start=(kt == 0), stop=(kt == KT - 1),
                skip_group_check=True,
            )


_NC_CACHE = None


def kernel(x, y, Wq, Wk, Wv):
    global _NC_CACHE
    if _NC_CACHE is None:
        _NC_CACHE = _build()
    nc = _NC_CACHE

    x = np.ascontiguousarray(np.asarray(x, dtype=np.float32))
    y = np.ascontiguousarray(np.asarray(y, dtype=np.float32))
    Wq = np.ascontiguousarray(np.asarray(Wq, dtype=np.float32))
    Wk = np.ascontiguousarray(np.asarray(Wk, dtype=np.float32))
    Wv = np.ascontiguousarray(np.asarray(Wv, dtype=np.float32))

    in_maps = [
        {"x": x[i * Q:(i + 1) * Q], "y": y, "Wq": Wq, "Wk": Wk, "Wv": Wv}
        for i in range(N_CORES)
    ]
    res = run_bass_kernel_spmd(nc, in_maps, core_ids=list(range(N_CORES)))
    return np.concatenate([res.results[i]["out"] for i in range(N_CORES)], axis=0)


# revision 9
# speedup vs baseline: 2.6915x; 2.6915x over previous
"""Trainium2 Bass kernel for nn_AttentionBlock (8-core SPMD, query-row sharded).

Reference (per core, q = 2048 rows of x):
  XQ = x @ Wq; YK = y @ Wk; YV = y @ Wv
  S = (XQ @ YK^T) / 16;  A = (0.1*relu(S) + softmax(S)) / rowsum(...)
  out = A @ YV

Key algebra (layout B: keys on partitions; no max subtraction — scores are
~N(0,1) so exp never overflows):
  S^T = y @ P8            with P8 = Wk @ XQ^T        (rank-7 contraction)
  U = exp(S^T/16), V = 0.1*relu(S^T/16)
  H1 = U^T @ Y8, H2 = V^T @ Y8   with Y8 = [y | 1 | 0pad]   (rank-8 stationary)
  G1 = H1 @ Wvo8, G2 = H2 @ Wvo8 with Wvo8 = [[Wv, 0], [0, 1], [0pad]]
  Z = G1[:, 256]; D = G2[:, 256] + 1
  out = (G1[:, :256]/Z + G2[:, :256]) / D
"""

import numpy as np

import concourse.bass as bass
import concourse.mybir as mybir
import concourse.tile as tile
from concourse import bacc
from concourse.bass_utils import run_bass_kernel_spmd
from concourse.masks import make_identity

P = 128
N_CORES = 8
N_FULL, M_CTX, SIN, YDIM, SPROJ = 16384, 4096, 256, 7, 256
Q = N_FULL // N_CORES          # 2048 query rows per core
QT = Q // P                    # 16 q-tiles
KT = M_CTX // P                # 32 k-tiles
CC = SPROJ // P                # 2 contraction chunks (proj dim)
QB = 256                       # q-block width in the main loop
NQB = Q // QB                  # q-blocks
SCALE = 1.0 / 16.0             # 1/sqrt(SPROJ)
R8 = 32                        # rank dim padded to 32 (ISA-friendly shapes)
GW = SPROJ + 2                 # G free width (257 used + 1 pad for even size)

MM_MODE = "f32r"               # "f32" | "f32r" | "bf16"

F32 = mybir.dt.float32
RDT = {
    "f32": F32,
    "f32r": mybir.dt.float32r,
    "bf16": mybir.dt.bfloat16,
}[MM_MODE]


def _build():
    nc = bacc.Bacc(
        "TRN2",
        target_bir_lowering=False,
        debug=False,
        num_devices=N_CORES,
    )
    x_d = nc.dram_tensor("x", [Q, SIN], F32, kind="ExternalInput").ap()
    y_d = nc.dram_tensor("y", [M_CTX, YDIM], F32, kind="ExternalInput").ap()
    wq_d = nc.dram_tensor("Wq", [SIN, SPROJ], F32, kind="ExternalInput").ap()
    wk_d = nc.dram_tensor("Wk", [YDIM, SPROJ], F32, kind="ExternalInput").ap()
    wv_d = nc.dram_tensor("Wv", [YDIM, SPROJ], F32, kind="ExternalInput").ap()
    out_d = nc.dram_tensor("out", [Q, SPROJ], F32, kind="ExternalOutput").ap()

    with tile.TileContext(nc) as tc:
        _body(tc, x_d, y_d, wq_d, wk_d, wv_d, out_d)
    nc.compile()
    return nc


def _body(tc, x_d, y_d, wq_d, wk_d, wv_d, out_d):
    nc = tc.nc
    Exp = mybir.ActivationFunctionType.Exp

    with tc.tile_pool(name="persist", bufs=1) as persist:
        # persistent main-loop operands (all in matmul dtype RDT)
        yTr = persist.tile([P, M_CTX], RDT, tag="yTr")       # y^T  [7(pad128), k]
        y8r = persist.tile([P, KT, R8], RDT, tag="y8r")      # [y|1|0] per k-tile
        p8r = persist.tile([P, Q], RDT, tag="p8r")           # Wk@XQ^T [7(pad128), q]
        wvo8r = persist.tile([R8, GW], RDT, tag="wvo8r")     # [[Wv,0],[0,1],[0]]

        # ---------------- preamble ----------------
        with (
            tc.tile_pool(name="pre", bufs=2) as pre,
            tc.tile_pool(name="pre_ps", bufs=2, space="PSUM") as pre_ps,
        ):
            ident = pre.tile([P, P], F32, tag="ident")
            make_identity(nc, ident)

            wq_sb = pre.tile([P, CC, SPROJ], F32, tag="wq")
            nc.sync.dma_start(wq_sb[:], wq_d.rearrange("(o p) f -> p o f", p=P))
            wqr = pre.tile([P, CC, SPROJ], RDT, tag="wqr")
            nc.vector.tensor_copy(wqr[:], wq_sb[:])

            wk_sb = pre.tile([P, SPROJ], F32, tag="wk")
            nc.vector.memset(wk_sb[:], 0.0)
            nc.sync.dma_start(wk_sb[:YDIM, :], wk_d)

            # Wvo8 [32, 258]: rows 0-6 = Wv, [7, 256] = 1, rest 0
            wvo8_f = pre.tile([R8, GW], F32, tag="wvo8f")
            nc.vector.memset(wvo8_f[:], 0.0)
            nc.sync.dma_start(wvo8_f[:YDIM, :SPROJ], wv_d)
            one_c = nc.inline_tensor(np.ones((1, 1), np.float32), name="one_c")
            nc.sync.dma_start(wvo8_f[YDIM:YDIM + 1, SPROJ:SPROJ + 1], one_c.ap())
            nc.vector.tensor_copy(wvo8r[:], wvo8_f[:])

            y_sb = pre.tile([P, KT, YDIM], F32, tag="y")
            nc.sync.dma_start(y_sb[:], y_d.rearrange("(o p) f -> p o f", p=P))

            # Y8 = [y | 1 | 0pad] per k-tile
            y8_f = pre.tile([P, KT, R8], F32, tag="y8f")
            nc.vector.memset(y8_f[:], 0.0)
            nc.vector.tensor_copy(y8_f[:, :, :YDIM], y_sb[:])
            nc.vector.memset(y8_f[:, :, YDIM:YDIM + 1], 1.0)
            nc.vector.tensor_copy(y8r[:], y8_f[:])

            # y^T [7(pad128), 4096] via PE transposes
            yT_f = pre.tile([P, M_CTX], F32, tag="yTf")
            nc.vector.memset(yT_f[:], 0.0)
            for o in range(KT):
                ps = pre_ps.tile([P, P], F32, tag="tps")
                nc.tensor.transpose(ps[:YDIM, :], y_sb[:, o, :], ident)
                nc.vector.tensor_copy(yT_f[:YDIM, o * P:(o + 1) * P], ps[:YDIM, :])
            nc.vector.tensor_copy(yTr[:], yT_f[:])

            # Wk^T chunks [c-chunk 128, 32] via PE transposes (cols 7-31 zero)
            wkT_f = pre.tile([P, CC, R8], F32, tag="wkTf")
            nc.vector.memset(wkT_f[:], 0.0)
            for c in range(CC):
                ps = pre_ps.tile([P, P], F32, tag="tps")
                nc.tensor.transpose(ps, wk_sb[:, c * P:(c + 1) * P], ident)
                nc.vector.tensor_copy(wkT_f[:, c, :YDIM], ps[:, :YDIM])
            wkTr = pre.tile([P, CC, R8], RDT, tag="wkTr")
            nc.vector.tensor_copy(wkTr[:], wkT_f[:])

            # x tiles -> x^T chunks via PE transposes
            x_sb = pre.tile([P, QT, SIN], F32, tag="x")
            nc.sync.dma_start(x_sb[:], x_d.rearrange("(o p) f -> p o f", p=P))
            xTr = pre.tile([P, CC, Q], RDT, tag="xTr")
            for t in range(QT):
                for c in range(CC):
                    ps = pre_ps.tile([P, P], F32, tag="tps")
                    nc.tensor.transpose(ps, x_sb[:, t, c * P:(c + 1) * P], ident)
                    nc.vector.tensor_copy(xTr[:, c, t * P:(t + 1) * P], ps)

            # XQ^T chunks [p-chunk 128, q], accumulate over SIN chunks
            xqTr = pre.tile([P, CC, Q], RDT, tag="xqTr")
            for pj in range(CC):
                for qb4 in range(Q // 512):
                    ps = pre_ps.tile([P, 512], F32, tag="mmps")
                    for ci in range(CC):
                        nc.tensor.matmul(
                            ps,
                            lhsT=wqr[:, ci, pj * P:(pj + 1) * P],
                            rhs=xTr[:, ci, qb4 * 512:(qb4 + 1) * 512],
                            start=(ci == 0), stop=(ci == CC - 1),
                        )
                    nc.vector.tensor_copy(xqTr[:, pj, qb4 * 512:(qb4 + 1) * 512], ps)

            # P8 = Wk @ XQ^T  [7(pad128), q]  (staged f32, single rounding copy)
            p8_f = pre.tile([P, Q], F32, tag="p8f")
            nc.vector.memset(p8_f[:], 0.0)
            for qb4 in range(Q // 512):
                ps = pre_ps.tile([P, 512], F32, tag="mmps")
                for ci in range(CC):
                    nc.tensor.matmul(
                        ps[:R8, :],
                        lhsT=wkTr[:, ci, :],
                        rhs=xqTr[:, ci, qb4 * 512:(qb4 + 1) * 512],
                        start=(ci == 0), stop=(ci == CC - 1),
                    )
                nc.vector.tensor_copy(
                    p8_f[:YDIM, qb4 * 512:(qb4 + 1) * 512], ps[:YDIM, :]
                )
            nc.vector.tensor_copy(p8r[:], p8_f[:])

        # ---------------- main loop ----------------
        with (
            tc.tile_pool(name="spool", bufs=3, space="PSUM") as spool,
            tc.tile_pool(name="hpool", bufs=1, space="PSUM") as hpool,
            tc.tile_pool(name="gpool", bufs=1, space="PSUM") as gpool,
            tc.tile_pool(name="uv", bufs=3) as uvpool,
            tc.tile_pool(name="epi", bufs=2) as epi,
        ):
            for qb in range(NQB):
                q0 = qb * QB
                h1 = hpool.tile([R8, QB], F32, tag="h1", name=f"h1_{qb}")
                h2 = hpool.tile([R8, QB], F32, tag="h2", name=f"h2_{qb}")

                prev_uv = None
                for kt in range(KT):
                    ps_s = spool.tile([P, QB], F32, tag="s")
                    nc.tensor.matmul(
                        ps_s,
                        lhsT=yTr[:, kt * P:(kt + 1) * P],
                        rhs=p8r[:, q0:q0 + QB],
                        start=True, stop=True,
                    )
                    if prev_uv is not None:
                        _av_matmuls(nc, h1, h2, y8r, prev_uv, kt - 1)

                    u = uvpool.tile([P, QB], RDT, tag="u")
                    nc.scalar.activation(u[:], ps_s[:], Exp, scale=SCALE)
                    v = uvpool.tile([P, QB], RDT, tag="v")
                    nc.vector.tensor_scalar(
                        v[:], ps_s[:], 0.1 * SCALE, 0.0,
                        mybir.AluOpType.mult, mybir.AluOpType.max,
                    )
                    prev_uv = (u, v)

                _av_matmuls(nc, h1, h2, y8r, prev_uv, KT - 1)

                # round H to matmul dtype (rows 8-31 are exact zeros from the
                # zero-padded Y8 columns)
                hs1r = epi.tile([R8, QB], RDT, tag="hs1r")
                nc.vector.tensor_copy(hs1r[:], h1[:])
                hs2r = epi.tile([R8, QB], RDT, tag="hs2r")
                nc.vector.tensor_copy(hs2r[:], h2[:])

                for qs in range(QB // P):
                    g1 = gpool.tile([P, GW], F32, tag="g1", name=f"g1_{qb}_{qs}")
                    nc.tensor.matmul(
                        g1, lhsT=hs1r[:, qs * P:(qs + 1) * P], rhs=wvo8r[:],
                        start=True, stop=True,
                    )
                    g2 = gpool.tile([P, GW], F32, tag="g2", name=f"g2_{qb}_{qs}")
                    nc.tensor.matmul(
                        g2, lhsT=hs2r[:, qs * P:(qs + 1) * P], rhs=wvo8r[:],
                        start=True, stop=True,
                    )

                    zinv = epi.tile([P, 1], F32, tag="zinv")
                    nc.vector.reciprocal(zinv[:], g1[:, SPROJ:SPROJ + 1])
                    dp1 = epi.tile([P, 1], F32, tag="dp1")
                    nc.vector.tensor_scalar_add(dp1[:], g2[:, SPROJ:SPROJ + 1], 1.0)
                    dinv = epi.tile([P, 1], F32, tag="dinv")
                    nc.vector.reciprocal(dinv[:], dp1[:])

                    acc = epi.tile([P, SPROJ], F32, tag="acc")
                    nc.vector.tensor_scalar_mul(acc[:], g1[:, :SPROJ], zinv[:])
                    nc.vector.tensor_tensor(
                        acc[:], acc[:], g2[:, :SPROJ], mybir.AluOpType.add
                    )
                    out_t = epi.tile([P, SPROJ], F32, tag="out")
                    nc.vector.tensor_scalar_mul(out_t[:], acc[:], dinv[:])
                    r0 = q0 + qs * P
                    nc.sync.dma_start(out_d[r0:r0 + P, :], out_t[:])


def _av_matmuls(nc, h1, h2, y8r, uv, kt):
    u, v = uv
    nc.tensor.matmul(
        h1[:], lhsT=y8r[:, kt, :], rhs=u[:],
        start=(kt == 0), stop=(kt == KT - 1), skip_group_check=True,
    )
    nc.tensor.matmul(
        h2[:], lhsT=y8r[:, kt, :], rhs=v[:],
        start=(kt == 0), stop=(kt == KT - 1), skip_group_check=True,
    )


_NC_CACHE = None


def kernel(x, y, Wq, Wk, Wv):
    global _NC_CACHE
    if _NC_CACHE is None:
        _NC_CACHE = _build()
    nc = _NC_CACHE

    x = np.ascontiguousarray(np.asarray(x, dtype=np.float32))
    y = np.ascontiguousarray(np.asarray(y, dtype=np.float32))
    Wq = np.ascontiguousarray(np.asarray(Wq, dtype=np.float32))
    Wk = np.ascontiguousarray(np.asarray(Wk, dtype=np.float32))
    Wv = np.ascontiguousarray(np.asarray(Wv, dtype=np.float32))

    in_maps = [
        {"x": x[i * Q:(i + 1) * Q], "y": y, "Wq": Wq, "Wk": Wk, "Wv": Wv}
        for i in range(N_CORES)
    ]
    res = run_bass_kernel_spmd(nc, in_maps, core_ids=list(range(N_CORES)))
    return np.concatenate([res.results[i]["out"] for i in range(N_CORES)], axis=0)
